# revision 16
# baseline (speedup 1.0000x reference)
"""BERT-style self-attention for Trainium2, data-parallel over batch (8 cores).

Problem: B=8, S=512, H=768, NH=12, HD=64.
Each core handles one batch element end-to-end (no collectives):
  q = h @ Wq.T + bq ; k = h @ Wk.T + bk ; v = h @ Wv.T + bv
  scores = q k^T / 8 + mask ; probs = softmax(scores) ; ctx = probs v

Dataflow: scores are computed TRANSPOSED (k on partitions, q on free dim)
so softmax needs no transposes:
  E[k, q]  = exp(scoresT * scale + mask[k])     (mask = per-partition bias)
  ctxT/Z   = (v_ext)^T @ E, v_ext = [64*v | 64] (one matmul gives both the
             unnormalized context AND the softmax denominator Z)
  ctx[q,d] = PE-transpose(ctxT) * (1/(64 Z))    (normalization at the end)

Schedule highlights:
- Inputs stream as fp8(e4m3) and are cast to fp16 in-flight by SWDGE DMA,
  halving the load time; weights are pre-scaled x64 on the host so e4m3's
  dynamic range is used well (compensated exactly in the exp scale and the
  v_ext ones column).
- q/k projection chunk c is immediately followed by the first two k-tiles
  of scores for heads (2c, 2c+1); the other two k-tiles interleave with the
  v projection. This paces scores production to the ACT engine's exp drain
  rate so the in-order PE queue never stalls on score PSUM banks, and the
  ~28us of exp work fully overlaps projection matmuls.
- PE warm-up matmuls cover the DMA window so HAM un-throttles the PE clock
  (1.2 -> 2.4 GHz) before real work.
- Per-head epilogue: 4 q-tile transposes land in ONE PSUM bank, then one
  reciprocal + one broadcast multiply normalize the whole head.
- Output staged fp16, pair-major p-contiguous DRAM layout (1KB DMA lines).
"""

import os
import sys

for _p in ("/opt/trn_rl_repo", "/root/.axon_site/_ro/trn_rl_repo"):
    if os.path.isdir(_p) and _p not in sys.path:
        sys.path.insert(0, _p)

import numpy as np
import ml_dtypes

from concourse import bacc, bass, tile
import concourse.mybir as mybir
from concourse.bass_utils import run_bass_kernel_spmd
from concourse.masks import make_identity

B, S, H, NH = 8, 512, 768, 12
HD = H // NH  # 64
P = 128
NC_ = H // P        # 6 feature chunks of 128
NS = S // P         # 4 sequence tiles of 128
HE = HD + 1         # 65: head dim + Z column
F32 = mybir.dt.float32

IN_DT = mybir.dt.float16      # on-chip matmul dtype
NP_IN = np.float16
# DRAM storage dtype; fp8 halves load time but costs ~3.7e-2 rel err (fails
# the 2e-2 gate), so default is fp16 (no cast -> HWDGE single queue).
LOAD = os.environ.get("KERNEL_LOAD", "fp16")
LD_DT = {"fp16": mybir.dt.float16, "fp8": mybir.dt.float8e4}[LOAD]
NP_LD = {"fp16": np.float16, "fp8": ml_dtypes.float8_e4m3}[LOAD]
WSCALE = {"fp16": 1.0, "fp8": 64.0}[LOAD]

WARM = int(os.environ.get("KERNEL_WARM", "20"))
TILEPOS = os.environ.get("KERNEL_TILEPOS", "0") == "1"


def build_nc():
    nc = bacc.Bacc(None, target_bir_lowering=False, debug=False)

    # ---- DRAM parameters (per-core views prepared on host) ----
    # hT: [768, 512] = hidden[b].T (fp8)
    # wqB/wkB: [6, 128, 768] output-block-major (x64): wB[oc, p, ic*128+c]
    #          = 64*W.T[ic*128+p, oc*128+c]
    # wvT: [768, 768] = 64*Wv.T (fp8); bv_r: [1, 768] = 64*bv (f32)
    # bq_pt/bk_pt: [128, 6] = 64*bias per (partition, out chunk)
    # mask_pt: [128, 4] additive mask per (partition, k-tile)
    # out: [6, 128, 4, 128] pair-major, p-contiguous fp16
    hT = nc.declare_dram_parameter("hT", [P, NC_ * S], LD_DT, isOutput=False)
    wqB = nc.declare_dram_parameter("wqB", [NC_, P, H], LD_DT, isOutput=False)
    wkB = nc.declare_dram_parameter("wkB", [NC_, P, H], LD_DT, isOutput=False)
    wvT = nc.declare_dram_parameter("wvT", [P, NC_ * H], LD_DT, isOutput=False)
    ones_r = nc.declare_dram_parameter("ones_r", [1, S], LD_DT, isOutput=False)
    bv_r = nc.declare_dram_parameter("bv_r", [1, H], LD_DT, isOutput=False)
    cst = nc.declare_dram_parameter(
        "consts_pt", [P, 2 * NC_ + NS], F32, isOutput=False)
    out = nc.declare_dram_parameter(
        "out", [NH // 2, P, NS, 2 * HD], IN_DT, isOutput=True)

    with tile.TileContext(nc) as tc:
        with (
            tc.tile_pool(name="consts", bufs=1) as consts,
            tc.tile_pool(name="inp", bufs=1) as inp,
            tc.tile_pool(name="qk", bufs=1) as qk,
            tc.tile_pool(name="cxp", bufs=2) as cxp,
            tc.tile_pool(name="outp", bufs=1) as outp,
            tc.tile_pool(name="rpool", bufs=2) as rpool,
            tc.tile_pool(name="proj_ps", bufs=2, space="PSUM") as proj_ps,
            tc.tile_pool(name="sc_ps", bufs=4, space="PSUM") as sc_ps,
            tc.tile_pool(name="ctx_ps", bufs=2, space="PSUM") as ctx_ps,
        ):
            cst_sb = consts.tile([P, 2 * NC_ + NS], F32)
            bq_sb = cst_sb[:, 0:NC_]
            bk_sb = cst_sb[:, NC_:2 * NC_]
            mask_sb = cst_sb[:, 2 * NC_:]

            # ---- loads spread over all three DMA queues in need-order;
            # each HWDGE dma_start costs ~0.6us of serialized sequencer
            # trigger time, so the first-needed tensors go on separate queues
            hT_sb = inp.tile([P, NC_, S], IN_DT)
            wq_sb = inp.tile([P, NC_, H], IN_DT)   # [p, oc, ic*128+c]
            wk_sb = inp.tile([P, NC_, H], IN_DT)
            wv_sb = inp.tile([P, NC_, H], IN_DT)   # [p, ic, oc cols]
            hT_ones = inp.tile([1, S], IN_DT)
            wv_bias = inp.tile([1, H], IN_DT)
            hs = hT_sb[:].rearrange("p c s -> p (c s)")
            nc.sync.dma_start(out=hs[:, 0:2 * S], in_=hT[:, 0:2 * S])
            nc.scalar.dma_start(out=wq_sb[:, 0, :], in_=wqB[0])
            nc.gpsimd.dma_start(out=wk_sb[:, 0, :], in_=wkB[0])
            nc.sync.dma_start(out=hs[:, 2 * S:6 * S], in_=hT[:, 2 * S:6 * S])
            nc.scalar.dma_start(out=cst_sb[:], in_=cst[:])
            nc.sync.dma_start(out=wq_sb[:, 1, :], in_=wqB[1])
            nc.scalar.dma_start(out=wk_sb[:, 1, :], in_=wkB[1])
            nc.sync.dma_start(
                out=wq_sb[:, 2:4, :],
                in_=wqB[2:4].rearrange("c p f -> p c f"))
            nc.scalar.dma_start(
                out=wk_sb[:, 2:4, :],
                in_=wkB[2:4].rearrange("c p f -> p c f"))
            nc.sync.dma_start(
                out=wq_sb[:, 4:6, :],
                in_=wqB[4:6].rearrange("c p f -> p c f"))
            nc.scalar.dma_start(
                out=wk_sb[:, 4:6, :],
                in_=wkB[4:6].rearrange("c p f -> p c f"))

            ident = consts.tile([P, P], IN_DT)
            make_identity(nc, ident)

            # ---- PE warm-up (HAM un-throttle during the DMA window) ----
            warm_in = consts.tile([P, P], IN_DT)
            nc.vector.memset(warm_in[:], 1.0)
            warm_ps = sc_ps.tile([P, P], F32, tag="sc")
            for _ in range(WARM):
                nc.tensor.matmul(warm_ps[:], warm_in[:], warm_in[:],
                                 start=True, stop=True)

            # ---- interleaved q/k projections + paced scores + exp ----
            qT = qk.tile([P, NC_, S], IN_DT)
            kT = qk.tile([P, NC_, S], IN_DT)
            E_all = qk.tile([P, NH, NS, S], IN_DT)  # exp(scoresT), persistent
            SC = 1.0 / (np.sqrt(HD) * WSCALE * WSCALE)

            def emit_score(h, kt):
                oc, off = h // 2, (h % 2) * HD
                ps = sc_ps.tile([P, S], F32, tag="sc")
                nc.tensor.matmul(
                    ps[:],
                    kT[off:off + HD, oc, kt * P:(kt + 1) * P],
                    qT[off:off + HD, oc, :],
                    start=True, stop=True,
                    tile_position=(off, 0) if TILEPOS else None,
                )
                nc.scalar.activation(
                    E_all[:, h, kt, :], ps[:],
                    mybir.ActivationFunctionType.Exp,
                    bias=mask_sb[:, kt:kt + 1], scale=SC,
                )

            for oc in range(NC_):
                for dst, w_sb, b_sb in ((qT, wq_sb, bq_sb), (kT, wk_sb, bk_sb)):
                    ps = proj_ps.tile([P, S], F32, tag="proj")
                    for ic in range(NC_):
                        nc.tensor.matmul(
                            ps[:],
                            w_sb[:, oc, ic * P:(ic + 1) * P],
                            hT_sb[:, ic, :],
                            start=(ic == 0), stop=(ic == NC_ - 1),
                        )
                    nc.vector.tensor_scalar_add(
                        out=dst[:, oc, :], in0=ps[:], scalar1=b_sb[:, oc:oc + 1])
                # first two k-tiles of scores for heads (2oc, 2oc+1);
                # kt=2,3 interleave with the v projection (paces PE to ACT)
                for kt in range(2):
                    emit_score(2 * oc, kt)
                    emit_score(2 * oc + 1, kt)

            deferred = [(h, kt) for kt in range(2, NS) for h in range(NH)]
            DEFER_N = [3] * 8

            # wv load is bulk (1.15MB) needed only in the v phase; issuing it
            # at t=0 would steal early DMA bandwidth from the critical-path
            # weight blocks. A GpSimd read of an oc=1-produced tile delays
            # the SWDGE descriptor generation until the head is past.
            wv_gate = consts.tile([P, 1], IN_DT)
            nc.gpsimd.tensor_copy(out=wv_gate[:], in_=E_all[:, 2, 0, 0:1])
            nc.gpsimd.dma_start(out=wv_sb[:].rearrange("p c f -> p (c f)"),
                                in_=wvT[:])
            nc.gpsimd.dma_start(out=hT_ones[:], in_=ones_r[:])
            nc.gpsimd.dma_start(out=wv_bias[:], in_=bv_r[:])

            # ---- V projection into v_ext [s-tile, 12*(64+1)], ones=64 ----
            v_ext = qk.tile([P, NS, NH * HE], IN_DT)
            nc.vector.memset(
                v_ext[:].rearrange("p t (h e) -> p t h e", e=HE)[:, :, :, HD:HE],
                WSCALE)
            HHALF = H // 2  # 384-wide halves, 6 heads each

            def emit_v_group(st, half, scores):
                ps = proj_ps.tile([P, HHALF], F32, tag="proj")
                for ic in range(NC_):
                    nc.tensor.matmul(
                        ps[:],
                        hT_sb[:, ic, st * P:(st + 1) * P],
                        wv_sb[:, ic, half * HHALF:(half + 1) * HHALF],
                        start=(ic == 0), stop=False,
                    )
                # K=1 bias row: v += ones(s) * (64 bv)  (exact)
                nc.tensor.matmul(
                    ps[:],
                    hT_ones[:, st * P:(st + 1) * P],
                    wv_bias[:, half * HHALF:(half + 1) * HHALF],
                    start=False, stop=True,
                )
                # deferred scores keep ACT fed while v projects
                for hk in scores:
                    emit_score(*hk)
                dst = v_ext[:, st, half * 6 * HE:(half + 1) * 6 * HE]
                nc.vector.tensor_copy(
                    out=dst.rearrange("p (h e) -> p h e", e=HE)[:, :, 0:HD],
                    in_=ps[:].rearrange("p (h d) -> p h d", d=HD),
                )

            out_sb = outp.tile([P, NS, H], IN_DT)

            def emit_ctx_mm(h):
                # ctxT_ext [65, 512]: rows 0..63 = 64 v^T E, row 64 = 64 Z
                cps = ctx_ps.tile([HE, S], F32, tag="ctx")
                for kt in range(NS):
                    nc.tensor.matmul(
                        cps[:],
                        v_ext[:, kt, h * HE:(h + 1) * HE],
                        E_all[:, h, kt, :],
                        start=(kt == 0), stop=(kt == NS - 1),
                    )
                csb = cxp.tile([HE, S], IN_DT, tag="csb")
                nc.scalar.activation(
                    csb[:], cps[:], mybir.ActivationFunctionType.Copy)
                return csb

            def emit_epilogue(h, csb):
                # per-qt stride padded to 66 elements (4-byte PSUM alignment)
                tp = sc_ps.tile([P, NS, HE + 1], IN_DT, tag="sc")
                for qt in range(NS):
                    nc.tensor.transpose(
                        tp[:, qt, 0:HE], csb[:, qt * P:(qt + 1) * P],
                        ident[0:HE, 0:HE])
                rp = rpool.tile([P, NS, 1], F32, tag="rp")
                nc.vector.reciprocal(rp[:], tp[:, :, HD:HE])
                nc.vector.tensor_tensor(
                    out=out_sb[:, :, h * HD:(h + 1) * HD],
                    in0=tp[:, :, 0:HD],
                    in1=rp[:].broadcast_to([P, NS, HD]),
                    op=mybir.AluOpType.mult,
                )

            def emit_out_dma(hp):
                c0 = 2 * hp * HD
                nc.sync.dma_start(
                    out=out[hp], in_=out_sb[:, :, c0:c0 + 2 * HD])

            # deferred scores head-major so early heads' E complete first
            dd = [(h, kt) for h in range(NH) for kt in range(2, NS)]
            di = 0
            for st in range(NS):
                for half in range(2):
                    emit_v_group(st, half, dd[di:di + 3])
                    di += 3
            for hp in range(NH // 2):
                csb0 = emit_ctx_mm(2 * hp)
                csb1 = emit_ctx_mm(2 * hp + 1)
                emit_epilogue(2 * hp, csb0)
                emit_epilogue(2 * hp + 1, csb1)
                emit_out_dma(hp)

    nc.compile()
    return nc


def _prep_inputs(hidden_states, attention_mask, Wq, bq, Wk, bk, Wv, bv):
    """Host-side shard + layout prep. Returns per-core input maps."""
    f32 = np.float32

    def blocks(w):  # [H,H] -> [oc, p, ic*128+c]; wB[oc,p,ic,c]=64*W.T[icP+p,ocP+c]
        wT = (np.asarray(w, f32).T * WSCALE).reshape(NC_, P, NC_, P)
        return np.ascontiguousarray(
            wT.transpose(2, 1, 0, 3).reshape(NC_, P, H)).astype(NP_LD)

    wqb = blocks(Wq)
    wkb = blocks(Wk)
    wvT = np.ascontiguousarray(
        (np.asarray(Wv, f32).T * WSCALE).reshape(NC_, P, H)
        .transpose(1, 0, 2).reshape(P, NC_ * H)).astype(NP_LD)
    ones_r = np.ones((1, S), f32).astype(NP_LD)
    bv_r = (np.asarray(bv, f32)[None, :] * WSCALE).astype(NP_LD)
    bq_pt = np.asarray(bq, f32).reshape(NC_, P).T * WSCALE
    bk_pt = np.asarray(bk, f32).reshape(NC_, P).T * WSCALE
    in_maps = []
    for b in range(B):
        hTb = np.ascontiguousarray(
            np.asarray(hidden_states[b], f32).T.reshape(NC_, P, S)
            .transpose(1, 0, 2).reshape(P, NC_ * S)).astype(NP_LD)
        mask_pt = np.asarray(
            attention_mask[b, 0, 0, :], f32).reshape(NS, P).T
        consts_pt = np.ascontiguousarray(
            np.concatenate([bq_pt, bk_pt, mask_pt], axis=1), dtype=f32)
        in_maps.append({
            "hT": hTb, "wqB": wqb, "wkB": wkb, "wvT": wvT,
            "ones_r": ones_r, "bv_r": bv_r, "consts_pt": consts_pt,
        })
    return in_maps


def _unshard_out(res):
    # out[b]: [6, 128, 4, 128] pair-major p-contiguous -> [512, 768]
    outs = []
    for b in range(B):
        o = np.asarray(res.results[b]["out"])
        o = o.transpose(2, 1, 0, 3).reshape(S, H)
        outs.append(o.astype(np.float32))
    return np.stack(outs, axis=0)


_NC_CACHE = None


def _install_ntff_hook():
    """Provide antenv.axon_hooks.get_axon_ntff_profile_hook via ctypes on
    libaxon_pjrt.so (the image's antenv stub lacks the submodule)."""
    import contextlib
    import ctypes
    import types

    try:
        import antenv.axon_hooks  # noqa: F401
        return True
    except ImportError:
        pass
    so_path = "/opt/axon/libaxon_pjrt.so"
    if not os.path.exists(so_path):
        return False
    lib = ctypes.CDLL(so_path)
    if not hasattr(lib, "axon_start_nrt_profile"):
        return False
    lib.axon_start_nrt_profile.argtypes = [
        ctypes.POINTER(ctypes.c_int64), ctypes.c_size_t]
    lib.axon_start_nrt_profile.restype = ctypes.c_int64
    lib.axon_stop_nrt_profile.argtypes = [ctypes.c_char_p]
    lib.axon_stop_nrt_profile.restype = ctypes.c_int64

    @contextlib.contextmanager
    def _hook(output_dir, device_ids):
        import jax
        jax.devices()
        if device_ids:
            ids = (ctypes.c_int64 * len(device_ids))(*device_ids)
            rc = lib.axon_start_nrt_profile(ids, len(device_ids))
        else:
            rc = lib.axon_start_nrt_profile(None, 0)
        if rc != 0:
            raise RuntimeError(f"axon_start_nrt_profile rc={rc}")
        try:
            yield
        finally:
            n = lib.axon_stop_nrt_profile(str(output_dir).encode())
            print(f"ntff profile: {n} file(s) -> {output_dir}", file=sys.stderr)

    import antenv
    mod = types.ModuleType("antenv.axon_hooks")
    mod.get_axon_ntff_profile_hook = lambda: _hook
    mod.set_axon_ntff_profile_hook = lambda h: None
    sys.modules["antenv.axon_hooks"] = mod
    antenv.axon_hooks = mod
    return True


def run(trace=False, tmpdir=None, **inputs):
    global _NC_CACHE
    if _NC_CACHE is None:
        _NC_CACHE = build_nc()
    if trace:
        trace = _install_ntff_hook()
    in_maps = _prep_inputs(**inputs)
    res = run_bass_kernel_spmd(
        _NC_CACHE, in_maps, list(range(B)), trace=trace, tmpdir=tmpdir)
    return _unshard_out(res), res


def kernel(**inputs):
    out, _ = run(trace=False, **inputs)
    return out


if __name__ == "__main__":
    rng = np.random.default_rng(0)
    hs = rng.standard_normal((B, S, H)).astype(np.float32)
    am = np.zeros((B, 1, 1, S), np.float32)
    mk = lambda: (rng.standard_normal((H, H)).astype(np.float32) * 0.02)
    o = kernel(hidden_states=hs, attention_mask=am,
               Wq=mk(), bq=np.zeros(H, np.float32),
               Wk=mk(), bk=np.zeros(H, np.float32),
               Wv=mk(), bv=np.zeros(H, np.float32))
    print(o.shape, o.dtype)
